# revision 19
# baseline (speedup 1.0000x reference)
"""BoundaryLoss Trainium2 kernel (8-core data-parallel).

loss = mean( (softplus(x) - t*x) * w ),  w = 1 + 5*boundary(t > 0.5)
boundary = dilate2(m) & ~erode2(m), 3x3 cross SE, 2 iterations, zero pad.

Reformulation: two iterations of cross erosion/dilation equal one
erosion/dilation by the L1-diamond of radius 2 (13 cells).  With S = the
13-cell sum of the binary mask m (zero padded):
    eroded = [S == 13], dilated = [S >= 1], boundary = [1 <= S <= 12]
    a = |5S - 32.5|  (= 32.5 iff S in {0,13}, else <= 27.5)
    v = max(a, 27.5) - 33.5 = -w          (one 4x-mode tensor_scalar)
    s' = t*x - softplus(x) = -bce
    q = v * s' = w * bce >= 0
sum(q) per tile is ONE native tensor_tensor_reduce (multiply + free-dim
row-sum fused), replacing the two accum_out passes of the previous
revision.  Halo rows are masked at the end by per-strip-kind row-
ownership vectors; the cross-core reduce is 8x128 floats on the host.

Per core: 4 images [1024,1024], split into 9 row-strips each (128 rows
loaded, owning 126/124/30 rows for the first/middle/tail strips; vertical
halo comes from the 2-row overlap, the top/bottom zero pad from band-matrix
truncation at partition edges).  Tiles pack two same-kind strips (an image
pair) side by side in the free dim (FD=2048).

Engines: S runs on the TensorEngine as 5 PSUM-accumulated band-matrix
matmuls per 512-col section (vertical reach via the band, horizontal reach
via column-shifted rhs windows, clipped at image edges = zero pad).
ScalarE does |5S-32.5| / exp / ln(1+e) from one activation-table set
(natural_log_exp_and_others).  Inputs are cast fp32->bf16 by the SWDGE
DMA (gpsimd-issued; descriptor generation rides the otherwise-idle POOL
engine), so DVE ops run in 2x/4x packed modes; on 8 of the 14 mid tiles
the mask compare also runs on POOL to shave the DVE critical path.
"""

import numpy as np
import ml_dtypes

import concourse.bass as bass
import concourse.mybir as mybir
import concourse.tile as tile
from concourse.bass_utils import run_bass_kernel_spmd

F32 = mybir.dt.float32
BF16 = mybir.dt.bfloat16
ALU = mybir.AluOpType
ACT = mybir.ActivationFunctionType

N_CORES = 8
B, H, W = 32, 1024, 1024
B_LOC = B // N_CORES            # 4 images per core


# ---------------------------------------------------------------------------
# Workaround: the neuronxcc walrus build encodes at most one sync-wait per
# instruction; Tile attaches several.  Split them onto single-wait NOPs on
# the same engine right before the instruction (engines execute in order).
def _patched_drain_and_barrier(self, tick_clock, wait_clock):
    from bass_rust import ScopedClock

    nc = self.nc
    probe = nc.sync.nop(hint="tile_tail_wait_probe")
    wait_clock.add_sem_waits(probe.ins, ScopedClock({None: tick_clock.global_clock}))
    waits = list(probe.ins.sync_info.on_wait or [])
    if waits:
        probe.ins.sync_info = mybir.SyncInfo(on_wait=[waits[0]], on_update=[])
        for w in waits[1:]:
            n = nc.sync.nop(hint="tile_tail_wait_split", nofuse=True)
            n.ins.sync_info = mybir.SyncInfo(on_wait=[w], on_update=[])
    nc.sync.drain()
    nc.all_engine_barrier()
    assert self.sems is not None
    popped = nc._tile_sem_poison_stack.pop()
    assert popped is self._sem_poison
    nc.clear_and_free_semaphores(list(self.sems.allocated().values()))
    nc.all_engine_barrier()


tile.TileContext._drain_and_barrier = _patched_drain_and_barrier


def _split_multi_waits(nc: bass.Bass) -> None:
    seen = set()
    nidx = 0
    for ctx in nc.bb_map.values():
        bb = ctx.bb
        if id(bb) in seen:
            continue
        seen.add(id(bb))
        insts = bb.instructions
        i = 0
        while i < len(insts):
            inst = insts[i]
            si = inst.sync_info
            if si is not None and si.on_wait and len(si.on_wait) > 1:
                waits = list(si.on_wait)
                for w in waits[:-1]:
                    nop = mybir.InstNoOp(name=f"I-waitsplit-{nidx}", ins=[], outs=[])
                    nidx += 1
                    nop.engine = inst.engine
                    nop.sync_info = mybir.SyncInfo(on_wait=[w], on_update=[])
                    nc.register_instruction(nop)
                    insts.insert(i, nop)
                    i += 1
                inst.sync_info = mybir.SyncInfo(
                    on_wait=[waits[-1]], on_update=list(si.on_update or [])
                )
            i += 1
# ---------------------------------------------------------------------------


def _band(width: int) -> np.ndarray:
    k = np.arange(128)
    return (np.abs(k[:, None] - k[None, :]) <= width).astype(ml_dtypes.bfloat16)


def _own(lo: int, hi: int) -> np.ndarray:
    v = np.zeros((128, 1), dtype=np.float32)
    v[lo:hi] = 1
    return v


# jobs: (kind, load_row, img_pair) — two same-kind strips per tile.
# "s0": rows 0..127 loaded, owns rows 0..125 (top pad = band truncation)
# "mid": rows a..a+127 loaded, owns a+2..a+125 (a = 124k, k=1..7)
# "tail": rows 992..1023 loaded (32 real, rest zeroed), owns 994..1023
# tail tiles LAST: their gpsimd memset chains must not delay the first loads.
_JOBS = (
    [("tail", 992, p) for p in ((0, 1), (2, 3))]
    + [("s0", 0, p) for p in ((0, 1), (2, 3))]
    + [
        ("mid", 124 * k, p)
        for p in ((0, 1), (2, 3))
        for k in range(1, 8)
    ]
)
_KIND_COLS = {"tail": (0, 2), "s0": (2, 4), "mid": (4, 18)}
_OWN_RANGES = {"s0": (0, 126), "mid": (2, 126), "tail": (2, 32)}


def build_nc(repeat: int = 1) -> bass.Bass:
    """repeat>1 wraps the tile loop in a HW For_i (timing variant)."""
    import contextlib

    nc = bass.Bass()

    x_d = nc.dram_tensor("inputs", [B_LOC, 1, H, W], F32, kind="ExternalInput")
    t_d = nc.dram_tensor("targets", [B_LOC, 1, H, W], F32, kind="ExternalInput")
    out_d = nc.dram_tensor("out", [128, 1], F32, kind="ExternalOutput")

    band_d = {w: nc.inline_tensor(_band(w), name=f"band{w}") for w in (0, 1, 2)}
    own_d = {k: nc.inline_tensor(_own(*r), name=f"own_{k}") for k, r in _OWN_RANGES.items()}

    n_jobs = len(_JOBS)
    terms = [(0, 2), (-1, 1), (1, 1), (-2, 0), (2, 0)]

    with tile.TileContext(nc) as tc:
        with (
            tc.tile_pool(name="const", bufs=1) as cpool,
            tc.tile_pool(name="acc", bufs=1) as apool,
            tc.tile_pool(name="work", bufs=6) as pool,
            tc.tile_pool(name="psum", bufs=2, space=bass.MemorySpace.PSUM) as psum,
        ):
            bands = {}
            for w in (0, 1, 2):
                bt = cpool.tile([128, 128], BF16, tag=f"band{w}")
                nc.sync.dma_start(bt[:], band_d[w][:])
                bands[w] = bt
            owns = {}
            for k, dten in own_d.items():
                ot = cpool.tile([128, 1], F32, tag=f"own_{k}")
                nc.sync.dma_start(ot[:], dten[:])
                owns[k] = ot
            bias_abs = cpool.tile([128, 1], F32, tag="bias_abs")
            nc.vector.memset(bias_abs[:], -32.5)

            acc = apool.tile([128, n_jobs], F32, tag="acc")
            nc.vector.memset(acc[:], 0.0)

            loop_ctx = tc.For_i(0, repeat, 1) if repeat > 1 else contextlib.nullcontext()
            with loop_ctx:
              for ti, (kind, row, pair) in enumerate(_JOBS):
                  t_t = pool.tile([128, 2 * W], BF16, tag="t")
                  x_t = pool.tile([128, 2 * W], BF16, tag="x")
                  m_t = pool.tile([128, 2 * W], BF16, tag="m")
                  a_t = pool.tile([128, 2 * W], BF16, tag="a")
                  e_t = pool.tile([128, 2 * W], BF16, tag="e")
                  sp_t = pool.tile([128, 2 * W], BF16, tag="sp")
                  tx_t = pool.tile([128, 2 * W], BF16, tag="tx")
                  s_t = e_t   # s' overwrites exp(x), dead after ln
                  v_t = x_t   # v overwrites x, dead after t*x
                  s_ps = psum.tile([128, 2 * W], F32, tag="S")

                  nrows = 32 if kind == "tail" else 128
                  for h, img in enumerate(pair):
                      fc = h * W
                      if nrows < 128:
                          nc.gpsimd.memset(t_t[:, fc : fc + W], 0.0)
                          nc.gpsimd.memset(x_t[:, fc : fc + W], 0.0)
                      nc.gpsimd.dma_start(
                          t_t[0:nrows, fc : fc + W], t_d[img, 0, row : row + nrows, :]
                      )
                      nc.gpsimd.dma_start(
                          x_t[0:nrows, fc : fc + W], x_d[img, 0, row : row + nrows, :]
                      )

                  # binary mask, both halves in one dense op
                  nc.vector.tensor_scalar(m_t[:], t_t[:], 0.5, None, ALU.is_gt)

                  # S = diamond-2 sum: 5 band matmuls per 512-col section,
                  # windows clipped at image columns (= zero padding)
                  for sec in range(4):
                      hbase = (sec // 2) * W
                      o = (sec % 2) * 512
                      for i, (dj, wd) in enumerate(terms):
                          c0 = max(o + dj, 0)
                          c1 = min(o + dj + 512, W)
                          outp = s_ps[:, sec * 512 + c0 - o - dj : sec * 512 + c1 - o - dj]
                          nc.tensor.matmul(
                              outp,
                              bands[wd][:],
                              m_t[:, hbase + c0 : hbase + c1],
                              start=(i == 0),
                              stop=(i == len(terms) - 1),
                          )

                  # bce tail: sp = ln(1+e^x).  ACT stream order is e, ln, a —
                  # e/ln depend only on the x DMA, so ACT never stalls on the
                  # matmuls; a comes last, right before its DVE consumers.
                  nc.scalar.activation(e_t[:], x_t[:], ACT.Exp)
                  nc.scalar.activation(sp_t[:], e_t[:], ACT.Ln, bias=1.0)

                  # s' = t*x - sp = -bce
                  nc.vector.tensor_tensor(tx_t[:], t_t[:], x_t[:], ALU.mult)
                  nc.vector.tensor_tensor(s_t[:], tx_t[:], sp_t[:], ALU.subtract)

                  # a = |5S - 32.5| ; v = max(a, 27.5) - 33.5 = -w
                  nc.scalar.activation(a_t[:], s_ps[:], ACT.Abs, bias=bias_abs[:], scale=5.0)
                  nc.vector.tensor_scalar(v_t[:], a_t[:], 27.5, -33.5, ALU.max, ALU.add)

                  # fused multiply + row-sum: q = (v*1)*s' = w*bce -> acc column
                  nc.vector.scalar_tensor_tensor(
                      m_t[:], v_t[:], 1.0, s_t[:], ALU.mult, ALU.mult,
                      accum_out=acc[:, ti : ti + 1],
                  )

            # out = sum_kind own_k * (kind-group row sums); halo rows masked
            out_t = apool.tile([128, 1], F32, tag="out")
            first = True
            for kd in ("s0", "mid", "tail"):
                lo, hi = _KIND_COLS[kd]
                rk = apool.tile([128, 1], F32, tag=f"rk_{kd}")
                nc.vector.tensor_reduce(rk[:], acc[:, lo:hi], mybir.AxisListType.X, ALU.add)
                if first:
                    nc.vector.tensor_scalar(out_t[:], rk[:], owns[kd][:], None, ALU.mult)
                    first = False
                else:
                    nc.vector.scalar_tensor_tensor(
                        out_t[:], rk[:], owns[kd][:], out_t[:], ALU.mult, ALU.add
                    )
            nc.sync.dma_start(out_d[:], out_t[:])

    _split_multi_waits(nc)
    nc.finalize()
    return nc


_NC = None


def _get_nc():
    global _NC
    if _NC is None:
        _NC = build_nc()
    return _NC


def kernel(inputs: np.ndarray, targets: np.ndarray) -> np.ndarray:
    nc = _get_nc()
    in_maps = [
        {
            "inputs": np.ascontiguousarray(inputs[c * B_LOC : (c + 1) * B_LOC]),
            "targets": np.ascontiguousarray(targets[c * B_LOC : (c + 1) * B_LOC]),
        }
        for c in range(N_CORES)
    ]
    res = run_bass_kernel_spmd(nc, in_maps, list(range(N_CORES)))
    total = sum(float(r["out"].sum()) for r in res.results)
    return np.float32(total / (B * H * W))


# revision 23
# speedup vs baseline: 1.0260x; 1.0260x over previous
"""BoundaryLoss Trainium2 kernel (8-core data-parallel).

loss = mean( (softplus(x) - t*x) * w ),  w = 1 + 5*boundary(t > 0.5)
boundary = dilate2(m) & ~erode2(m), 3x3 cross SE, 2 iterations, zero pad.

Reformulation: two iterations of cross erosion/dilation equal one
erosion/dilation by the L1-diamond of radius 2 (13 cells).  With S = the
13-cell sum of the binary mask m (zero padded):
    eroded = [S == 13], dilated = [S >= 1], boundary = [1 <= S <= 12]
    a = |5S - 32.5|  (= 32.5 iff S in {0,13}, else <= 27.5)
    v = max(a, 27.5) - 33.5 = -w          (one 4x-mode tensor_scalar)
    s' = t*x - softplus(x) = -bce
    q = v * s' = w * bce >= 0
sum(q) per tile is ONE native scalar_tensor_tensor with accum_out
(multiply + free-dim row-sum fused), replacing the two accum_out passes
of the previous revision.  Halo rows are masked at the end by per-strip-kind row-
ownership vectors; the cross-core reduce is 8x128 floats on the host.

Per core: 4 images [1024,1024], split into 9 row-strips each (128 rows
loaded, owning 126/124/30 rows for the first/middle/tail strips; vertical
halo comes from the 2-row overlap, the top/bottom zero pad from band-matrix
truncation at partition edges).  Tiles pack two same-kind strips (an image
pair) side by side in the free dim (FD=2048).

Engines: S runs on the TensorEngine as 5 PSUM-accumulated band-matrix
matmuls per 512-col section (vertical reach via the band, horizontal reach
via column-shifted rhs windows, clipped at image edges = zero pad).
ScalarE does |5S-32.5| / exp / ln(1+e) from one activation-table set
(natural_log_exp_and_others, preloaded during the DMA ramp by a dummy
[128,1] Exp); its stream is ordered e, ln, a so it never stalls on the
matmuls.  Inputs are cast fp32->bf16 by the SWDGE DMA (gpsimd-issued;
descriptor generation rides the otherwise-idle POOL engine), so DVE ops
run in 2x/4x packed modes.  Tail-tile zero-fills run on the DVE (idle
during the ramp) so POOL's first descriptor generation is never delayed.
Work pool bufs=6 with two tiles reused in place (s' over exp(x), v over
x) keeps ~6 tiles of DMA lookahead in SBUF.
"""

import numpy as np
import ml_dtypes

import concourse.bass as bass
import concourse.mybir as mybir
import concourse.tile as tile
from concourse.bass_utils import run_bass_kernel_spmd

F32 = mybir.dt.float32
BF16 = mybir.dt.bfloat16
ALU = mybir.AluOpType
ACT = mybir.ActivationFunctionType

N_CORES = 8
B, H, W = 32, 1024, 1024
B_LOC = B // N_CORES            # 4 images per core


# ---------------------------------------------------------------------------
# Workaround: the neuronxcc walrus build encodes at most one sync-wait per
# instruction; Tile attaches several.  Split them onto single-wait NOPs on
# the same engine right before the instruction (engines execute in order).
def _patched_drain_and_barrier(self, tick_clock, wait_clock):
    from bass_rust import ScopedClock

    nc = self.nc
    probe = nc.sync.nop(hint="tile_tail_wait_probe")
    wait_clock.add_sem_waits(probe.ins, ScopedClock({None: tick_clock.global_clock}))
    waits = list(probe.ins.sync_info.on_wait or [])
    if waits:
        probe.ins.sync_info = mybir.SyncInfo(on_wait=[waits[0]], on_update=[])
        for w in waits[1:]:
            n = nc.sync.nop(hint="tile_tail_wait_split", nofuse=True)
            n.ins.sync_info = mybir.SyncInfo(on_wait=[w], on_update=[])
    nc.sync.drain()
    nc.all_engine_barrier()
    assert self.sems is not None
    popped = nc._tile_sem_poison_stack.pop()
    assert popped is self._sem_poison
    nc.clear_and_free_semaphores(list(self.sems.allocated().values()))
    nc.all_engine_barrier()


tile.TileContext._drain_and_barrier = _patched_drain_and_barrier


def _split_multi_waits(nc: bass.Bass) -> None:
    seen = set()
    nidx = 0
    for ctx in nc.bb_map.values():
        bb = ctx.bb
        if id(bb) in seen:
            continue
        seen.add(id(bb))
        insts = bb.instructions
        i = 0
        while i < len(insts):
            inst = insts[i]
            si = inst.sync_info
            if si is not None and si.on_wait and len(si.on_wait) > 1:
                waits = list(si.on_wait)
                for w in waits[:-1]:
                    nop = mybir.InstNoOp(name=f"I-waitsplit-{nidx}", ins=[], outs=[])
                    nidx += 1
                    nop.engine = inst.engine
                    nop.sync_info = mybir.SyncInfo(on_wait=[w], on_update=[])
                    nc.register_instruction(nop)
                    insts.insert(i, nop)
                    i += 1
                inst.sync_info = mybir.SyncInfo(
                    on_wait=[waits[-1]], on_update=list(si.on_update or [])
                )
            i += 1
# ---------------------------------------------------------------------------


def _band(width: int) -> np.ndarray:
    k = np.arange(128)
    return (np.abs(k[:, None] - k[None, :]) <= width).astype(ml_dtypes.bfloat16)


def _own(lo: int, hi: int) -> np.ndarray:
    v = np.zeros((128, 1), dtype=np.float32)
    v[lo:hi] = 1
    return v


# jobs: (kind, load_row, img_pair) — two same-kind strips per tile.
# "s0": rows 0..127 loaded, owns rows 0..125 (top pad = band truncation)
# "mid": rows a..a+127 loaded, owns a+2..a+125 (a = 124k, k=1..7)
# "tail": rows 992..1023 loaded (32 real, rest zeroed), owns 994..1023
# tail tiles LAST: their gpsimd memset chains must not delay the first loads.
_JOBS = (
    [("tail", 992, p) for p in ((0, 1), (2, 3))]
    + [("s0", 0, p) for p in ((0, 1), (2, 3))]
    + [
        ("mid", 124 * k, p)
        for p in ((0, 1), (2, 3))
        for k in range(1, 8)
    ]
)
_KIND_COLS = {"tail": (0, 2), "s0": (2, 4), "mid": (4, 18)}
_OWN_RANGES = {"s0": (0, 126), "mid": (2, 126), "tail": (2, 32)}


def build_nc(repeat: int = 1) -> bass.Bass:
    """repeat>1 wraps the tile loop in a HW For_i (timing variant)."""
    import contextlib

    nc = bass.Bass()

    x_d = nc.dram_tensor("inputs", [B_LOC, 1, H, W], F32, kind="ExternalInput")
    t_d = nc.dram_tensor("targets", [B_LOC, 1, H, W], F32, kind="ExternalInput")
    out_d = nc.dram_tensor("out", [128, 1], F32, kind="ExternalOutput")

    band_d = {w: nc.inline_tensor(_band(w), name=f"band{w}") for w in (0, 1, 2)}
    own_d = {k: nc.inline_tensor(_own(*r), name=f"own_{k}") for k, r in _OWN_RANGES.items()}

    n_jobs = len(_JOBS)
    terms = [(0, 2), (-1, 1), (1, 1), (-2, 0), (2, 0)]

    with tile.TileContext(nc) as tc:
        with (
            tc.tile_pool(name="const", bufs=1) as cpool,
            tc.tile_pool(name="acc", bufs=1) as apool,
            tc.tile_pool(name="work", bufs=6) as pool,
            tc.tile_pool(name="psum", bufs=2, space=bass.MemorySpace.PSUM) as psum,
        ):
            bands = {}
            for w in (0, 1, 2):
                bt = cpool.tile([128, 128], BF16, tag=f"band{w}")
                nc.sync.dma_start(bt[:], band_d[w][:])
                bands[w] = bt
            owns = {}
            for k, dten in own_d.items():
                ot = cpool.tile([128, 1], F32, tag=f"own_{k}")
                nc.sync.dma_start(ot[:], dten[:])
                owns[k] = ot
            bias_abs = cpool.tile([128, 1], F32, tag="bias_abs")
            nc.vector.memset(bias_abs[:], -32.5)
            warm = cpool.tile([128, 1], F32, tag="act_warm")
            nc.scalar.activation(warm[:], bias_abs[:], ACT.Exp)

            acc = apool.tile([128, n_jobs], F32, tag="acc")
            nc.vector.memset(acc[:], 0.0)

            loop_ctx = tc.For_i(0, repeat, 1) if repeat > 1 else contextlib.nullcontext()
            with loop_ctx:
              for ti, (kind, row, pair) in enumerate(_JOBS):
                  t_t = pool.tile([128, 2 * W], BF16, tag="t")
                  x_t = pool.tile([128, 2 * W], BF16, tag="x")
                  m_t = pool.tile([128, 2 * W], BF16, tag="m")
                  a_t = pool.tile([128, 2 * W], BF16, tag="a")
                  e_t = pool.tile([128, 2 * W], BF16, tag="e")
                  sp_t = pool.tile([128, 2 * W], BF16, tag="sp")
                  tx_t = pool.tile([128, 2 * W], BF16, tag="tx")
                  s_t = e_t   # s' overwrites exp(x), dead after ln
                  v_t = x_t   # v overwrites x, dead after t*x
                  s_ps = psum.tile([128, 2 * W], F32, tag="S")

                  nrows = 32 if kind == "tail" else 128
                  for h, img in enumerate(pair):
                      fc = h * W
                      if nrows < 128:
                          nc.vector.memset(t_t[:, fc : fc + W], 0.0)
                          nc.vector.memset(x_t[:, fc : fc + W], 0.0)
                      nc.gpsimd.dma_start(
                          t_t[0:nrows, fc : fc + W], t_d[img, 0, row : row + nrows, :]
                      )
                      nc.gpsimd.dma_start(
                          x_t[0:nrows, fc : fc + W], x_d[img, 0, row : row + nrows, :]
                      )

                  # binary mask, both halves in one dense op
                  nc.vector.tensor_scalar(m_t[:], t_t[:], 0.5, None, ALU.is_gt)

                  # S = diamond-2 sum: 5 band matmuls per 512-col section,
                  # windows clipped at image columns (= zero padding)
                  for sec in range(4):
                      hbase = (sec // 2) * W
                      o = (sec % 2) * 512
                      for i, (dj, wd) in enumerate(terms):
                          c0 = max(o + dj, 0)
                          c1 = min(o + dj + 512, W)
                          outp = s_ps[:, sec * 512 + c0 - o - dj : sec * 512 + c1 - o - dj]
                          nc.tensor.matmul(
                              outp,
                              bands[wd][:],
                              m_t[:, hbase + c0 : hbase + c1],
                              start=(i == 0),
                              stop=(i == len(terms) - 1),
                          )

                  # bce tail: sp = ln(1+e^x).  ACT stream order is e, ln, a —
                  # e/ln depend only on the x DMA, so ACT never stalls on the
                  # matmuls; a comes last, right before its DVE consumers.
                  nc.scalar.activation(e_t[:], x_t[:], ACT.Exp)
                  nc.scalar.activation(sp_t[:], e_t[:], ACT.Ln, bias=1.0)

                  # s' = t*x - sp = -bce
                  nc.vector.tensor_tensor(tx_t[:], t_t[:], x_t[:], ALU.mult)
                  nc.vector.tensor_tensor(s_t[:], tx_t[:], sp_t[:], ALU.subtract)

                  # a = |5S - 32.5| ; v = max(a, 27.5) - 33.5 = -w
                  nc.scalar.activation(a_t[:], s_ps[:], ACT.Abs, bias=bias_abs[:], scale=5.0)
                  nc.vector.tensor_scalar(v_t[:], a_t[:], 27.5, -33.5, ALU.max, ALU.add)

                  # fused multiply + row-sum: q = (v*1)*s' = w*bce -> acc column
                  nc.vector.scalar_tensor_tensor(
                      m_t[:], v_t[:], 1.0, s_t[:], ALU.mult, ALU.mult,
                      accum_out=acc[:, ti : ti + 1],
                  )

            # out = sum_kind own_k * (kind-group row sums); halo rows masked
            out_t = apool.tile([128, 1], F32, tag="out")
            first = True
            for kd in ("s0", "mid", "tail"):
                lo, hi = _KIND_COLS[kd]
                rk = apool.tile([128, 1], F32, tag=f"rk_{kd}")
                nc.vector.tensor_reduce(rk[:], acc[:, lo:hi], mybir.AxisListType.X, ALU.add)
                if first:
                    nc.vector.tensor_scalar(out_t[:], rk[:], owns[kd][:], None, ALU.mult)
                    first = False
                else:
                    nc.vector.scalar_tensor_tensor(
                        out_t[:], rk[:], owns[kd][:], out_t[:], ALU.mult, ALU.add
                    )
            nc.sync.dma_start(out_d[:], out_t[:])

    _split_multi_waits(nc)
    nc.finalize()
    return nc


_NC = None


def _get_nc():
    global _NC
    if _NC is None:
        _NC = build_nc()
    return _NC


def kernel(inputs: np.ndarray, targets: np.ndarray) -> np.ndarray:
    nc = _get_nc()
    in_maps = [
        {
            "inputs": np.ascontiguousarray(inputs[c * B_LOC : (c + 1) * B_LOC]),
            "targets": np.ascontiguousarray(targets[c * B_LOC : (c + 1) * B_LOC]),
        }
        for c in range(N_CORES)
    ]
    res = run_bass_kernel_spmd(nc, in_maps, list(range(N_CORES)))
    total = sum(float(r["out"].sum()) for r in res.results)
    return np.float32(total / (B * H * W))
